# revision 40
# baseline (speedup 1.0000x reference)
"""Trainium2 Bass kernel for nn_Attention_LoRA_FFT.

Sharding: data-parallel over batch B=8 across the 8 NeuronCores. The DCT
LoRA weight reconstruction is sharded: each core builds a 256-column
slice of one of (W_qk + Wk).T / (W_qv + Wv).T -- the dense QKV weight
slice is merged in BEFORE the AllGather (host passes the per-core slice),
so the gathered weights are the final effective K/V projection weights
and kT / V' are single-contraction GEMMs.

All tensors bf16 (f32 PSUM accumulation); numerics sim puts rel-absmax
error at ~1.25e-2 vs the 2e-2 gate.

Per-core program (issue order = overlap plan):
  A) G = Sw.T @ Bmq ; Wpart_hf = Bm.T @ G_hf + Wqkv-slice (DVE add); the
     cc writeout + AllGather trigger for each half go on the gpsimd
     queue so they never wait behind input streams; readbacks trail.
  B) qT = W_q @ x.T (covers the collective window).
  C) kT(fc) single pass from merged weights; attention stage1 (QK+exp)
     fires per head-pair chunk (even chunks first: they only need
     gather half 0); V' (single pass, ones column for Z) interleaves;
     stage2 (PV + 1/Z) trails the exps; output projection interleaves
     as oT halves complete. ACT does ONLY the 128 exps; copies/adds
     live on DVE; broadcasts on gpsimd.
"""

import os
import sys

for _p in ("/opt/trn_rl_repo", "/root/.axon_site/_ro/trn_rl_repo"):
    if os.path.isdir(_p) and _p not in sys.path:
        sys.path.insert(0, _p)

import numpy as np

import concourse.bacc as bacc
import concourse.mybir as mybir
from concourse.tile import TileContext
from concourse.bass_utils import run_bass_kernel_spmd

B, N, C = 8, 1024, 1024
H, HD = 16, 64
NCORES = 8
PC = C // 128
F32 = mybir.dt.float32
BF16 = mybir.dt.bfloat16
EXP = mybir.ActivationFunctionType.Exp
SWM = 64  # max scatter nonzeros per (partition, column-chunk) bucket


def _dct_matrix(n: int) -> np.ndarray:
    i = np.arange(n, dtype=np.float64)[:, None]
    j = np.arange(n, dtype=np.float64)[None, :]
    m = np.sqrt(2.0 / n) * np.cos(np.pi * i * (2.0 * j + 1.0) / (2.0 * n))
    m[0, :] = np.sqrt(1.0 / n)
    return m.astype(np.float32)


def _build():
    nc = bacc.Bacc("TRN2", target_bir_lowering=False, debug=False, num_devices=NCORES)

    xT_d = nc.dram_tensor("xT", [C, N], BF16, kind="ExternalInput")
    wqkvT_d = nc.dram_tensor("wqkvT", [C, 3 * C], BF16, kind="ExternalInput")
    wprojT_d = nc.dram_tensor("wprojT", [C, C], BF16, kind="ExternalInput")
    bias_d = nc.dram_tensor("bias", [C, 1], F32, kind="ExternalInput")
    bm_d = nc.dram_tensor("bm", [C, C], BF16, kind="ExternalInput")
    swidx_d = nc.dram_tensor("swidx", [C, SWM], mybir.dt.int16, kind="ExternalInput")
    swval_d = nc.dram_tensor("swval", [C, SWM], BF16, kind="ExternalInput")
    bmq_d = nc.dram_tensor("bmq", [C, 256], BF16, kind="ExternalInput")
    wqsl_d = nc.dram_tensor("wqsl", [C, 256], BF16, kind="ExternalInput")
    yT_d = nc.dram_tensor("yT", [C, N], BF16, kind="ExternalOutput")
    # partition-major cc layout: row = source partition, cols contiguous —
    # descriptor-light DMA on both the writeout and the readback
    cc_in0 = nc.dram_tensor("cc_in0", [128, C], BF16)
    cc_in1 = nc.dram_tensor("cc_in1", [128, C], BF16)
    cc_out0 = nc.dram_tensor("cc_out0", [NCORES * 128, C], BF16, addr_space="Shared")
    cc_out1 = nc.dram_tensor("cc_out1", [NCORES * 128, C], BF16, addr_space="Shared")

    def col_slab(dram_ap, pool, tag, f0, width, eng=None):
        slab = pool.tile([128, PC, width], BF16, tag=tag, name=tag)
        (eng or nc.sync).dma_start(
            out=slab[:],
            in_=dram_ap[:, f0 : f0 + width].rearrange("(cc p) f -> p cc f", p=128),
        )
        return slab

    from concourse import library_config

    with TileContext(nc) as tc:
        # preload the scatter ucode lib so the first local_scatter doesn't
        # eat a lazy LOAD_LIB on the startup critical path
        nc.gpsimd.load_library(library_config.local_scatter)

        # ---------------- left stack (persistent) ----------------
        small_p = tc.alloc_tile_pool(name="small", bufs=1, side="left")
        bias_sb = small_p.tile([128, PC, 1], F32, tag="bias")
        nc.sync.dma_start(
            out=bias_sb[:], in_=bias_d.rearrange("(cc p) o -> p cc o", p=128)
        )

        x_p = tc.alloc_tile_pool(name="xp", bufs=1, side="left")
        wk_p = tc.alloc_tile_pool(name="wkp", bufs=1, side="left")
        wv_p = tc.alloc_tile_pool(name="wvp", bufs=1, side="left")
        wpj_p = tc.alloc_tile_pool(name="wpjp", bufs=1, side="left")
        x_sb = x_p.tile([128, PC, N], BF16, tag="x")
        wk_sb = wk_p.tile([128, PC, C], BF16, tag="wk")
        wv_sb = wv_p.tile([128, PC, C], BF16, tag="wv")
        wpj_sb = wpj_p.tile([128, PC, C], BF16, tag="wpj")
        # x + wproj stream on the scalar queue (idle until the exps)
        for cc in range(PC):
            nc.scalar.dma_start(
                out=x_sb[:, cc, :], in_=xT_d[cc * 128 : (cc + 1) * 128, :]
            )


        # ================= Phase A: sharded LoRA reconstruction =======
        sw_p = tc.alloc_tile_pool(name="swp", bufs=1, side="right")
        bm_p = tc.alloc_tile_pool(name="bmp", bufs=1, side="right")
        bmq_p = tc.alloc_tile_pool(name="bmqp", bufs=1, side="right")
        wqsl_p = tc.alloc_tile_pool(name="wqslp", bufs=1, side="right")
        g_p = tc.alloc_tile_pool(name="gp", bufs=1, side="right")
        wpart_p = tc.alloc_tile_pool(name="wpartp", bufs=1, side="right")
        psA = tc.alloc_tile_pool(name="psA", bufs=4, space="PSUM")

        # The scatter plane S arrives SPARSE (per-partition/chunk index +
        # value buckets, ~256KB) and is densified on gpsimd; kills the 2MB
        # dense stream from the startup critical path.
        sw_sb = sw_p.tile([128, PC, C], BF16, tag="sw")
        swidx_sb = sw_p.tile([128, PC, SWM], mybir.dt.int16, tag="swidx")
        swval_sb = sw_p.tile([128, PC, SWM], BF16, tag="swval")
        bm_sb = bm_p.tile([128, PC, C], BF16, tag="bm")
        bmq_sb = bmq_p.tile([128, PC, 256], BF16, tag="bmq")
        wqsl_sb = wqsl_p.tile([128, PC, 256], BF16, tag="wqsl")
        nc.sync.dma_start(
            out=swidx_sb[:], in_=swidx_d.rearrange("(cc p) m -> p cc m", p=128)
        )
        nc.sync.dma_start(
            out=swval_sb[:], in_=swval_d.rearrange("(cc p) m -> p cc m", p=128)
        )
        nc.sync.dma_start(
            out=bmq_sb[:], in_=bmq_d.rearrange("(cc p) f -> p cc f", p=128)
        )
        for cc in range(PC):
            nc.gpsimd.local_scatter(
                out_ap=sw_sb[:, cc, :],
                data_ap=swval_sb[:, cc, :],
                idxs_ap=swidx_sb[:, cc, :],
                channels=128,
                num_elems=C,
                num_idxs=SWM,
            )
        # switch to the attn lib (partition_broadcast) while gpsimd is idle
        nc.gpsimd.load_library(library_config.attn)

        nc.sync.dma_start(
            out=wqsl_sb[:], in_=wqsl_d.rearrange("(cc p) f -> p cc f", p=128)
        )
        g_sb = g_p.tile([128, PC, 256], BF16, tag="g", name="g_sb")
        wpart_sb = wpart_p.tile([128, PC, 256], BF16, tag="wpart", name="wpart_sb")
        for at in range(PC):
            ps = psA.tile([128, 256], F32, tag="psA", name="psA_t")
            for bc in range(PC):
                nc.tensor.matmul(
                    ps[:],
                    sw_sb[:, bc, at * 128 : (at + 1) * 128],
                    bmq_sb[:, bc, :],
                    start=(bc == 0),
                    stop=(bc == PC - 1),
                )
            nc.vector.tensor_copy(g_sb[:, at, :], ps[:])
        for ct in range(PC):
            nc.sync.dma_start(
                out=bm_sb[:, :, ct * 128 : (ct + 1) * 128],
                in_=bm_d[:, ct * 128 : (ct + 1) * 128].rearrange(
                    "(cc p) f -> p cc f", p=128
                ),
            )

        # W halves: Wpart_hf = Bm.T @ G_hf + Wqkv-slice, writeout + gather
        # triggers on the gpsimd queue (never behind input streams).
        for hf, cc_in in ((0, cc_in0), (1, cc_in1)):
            for ct in range(PC):
                ps = psA.tile([128, 256], F32, tag="psA", name="psA_t")
                for ac in range(PC):
                    nc.tensor.matmul(
                        ps[:, 0:128],
                        bm_sb[:, ac, ct * 128 : (ct + 1) * 128],
                        g_sb[:, ac, hf * 128 : (hf + 1) * 128],
                        start=(ac == 0),
                        stop=(ac == PC - 1),
                    )
                nc.vector.tensor_add(
                    wpart_sb[:, ct, hf * 128 : (hf + 1) * 128],
                    ps[:, 0:128],
                    wqsl_sb[:, ct, hf * 128 : (hf + 1) * 128],
                )
            nc.gpsimd.dma_start(
                out=cc_in.rearrange("p (ct f) -> p ct f", f=128),
                in_=wpart_sb[:, :, hf * 128 : (hf + 1) * 128],
            )
            nc.gpsimd.collective_compute(
                "AllGather",
                mybir.AluOpType.bypass,
                replica_groups=[list(range(NCORES))],
                ins=[cc_in[:]],
                outs=[cc_out0[:] if hf == 0 else cc_out1[:]],
            )
        # readbacks trail both triggers on the gpsimd queue
        for hf, cc_out in ((0, cc_out0), (1, cc_out1)):
            for wi, w_sb in ((0, wk_sb), (1, wv_sb)):
                for fq in range(4):
                    base = (wi * 4 + fq) * 128
                    nc.gpsimd.dma_start(
                        out=w_sb[
                            :, :, fq * 256 + hf * 128 : fq * 256 + (hf + 1) * 128
                        ],
                        in_=cc_out[base : base + 128, :].rearrange(
                            "p (ct f) -> p ct f", f=128
                        ),
                    )
        # wproj streams late on the gpsimd queue (needed only by the
        # projection) so it never competes with the startup streams
        for ct in range(PC):
            nc.gpsimd.dma_start(
                out=wpj_sb[:, :, ct * 128 : (ct + 1) * 128],
                in_=wprojT_d[:, ct * 128 : (ct + 1) * 128].rearrange(
                    "(cc p) f -> p cc f", p=128
                ),
            )

        psA.release()
        wpart_p.release()
        g_p.release()
        wqsl_p.release()
        bmq_p.release()
        bm_p.release()
        sw_p.release()

        # ================= Phase B + C pools ===========================
        qt_p = tc.alloc_tile_pool(name="qtp", bufs=1, side="right")
        kt_p = tc.alloc_tile_pool(name="ktp", bufs=1, side="right")
        vp_p = tc.alloc_tile_pool(name="vpp", bufs=1, side="right")
        ot_p = tc.alloc_tile_pool(name="otp", bufs=1, side="right")
        pt_p = tc.alloc_tile_pool(name="ptp", bufs=3, side="right")
        os_p = tc.alloc_tile_pool(name="osp", bufs=2, side="right")
        rz_p = tc.alloc_tile_pool(name="rzp", bufs=2, side="right")
        zb_p = tc.alloc_tile_pool(name="zbp", bufs=1, side="right")
        slabB_p = tc.alloc_tile_pool(name="slabB", bufs=3, side="right")
        y_p = tc.alloc_tile_pool(name="yp", bufs=2, side="right")
        psB = tc.alloc_tile_pool(name="psB", bufs=2, space="PSUM")
        psS = tc.alloc_tile_pool(name="psS", bufs=1, space="PSUM")
        psO = tc.alloc_tile_pool(name="psO", bufs=1, space="PSUM")

        qT_sb = qt_p.tile([128, PC, N], BF16, tag="qT")
        kT_sb = kt_p.tile([128, PC, N], BF16, tag="kT")
        vp_sb = vp_p.tile([128, PC, H, HD + 1], BF16, tag="vp")
        oT_sb = ot_p.tile([128, PC, N], BF16, tag="oT")
        # ones column for the softmax denominator
        for tc_i in range(PC):
            nc.vector.memset(vp_sb[:, tc_i, :, HD : HD + 1], 1.0)

        # ---- qT: no gather dependency, covers the collective ----
        for fc in range(PC):
            slab = col_slab(wqkvT_d, slabB_p, "slabB", fc * 128, 128)
            for th in range(2):
                ps = psB.tile([128, 512], F32, tag="psB", name="psB_t")
                for cc in range(PC):
                    nc.tensor.matmul(
                        ps[:],
                        slab[:, cc, :],
                        x_sb[:, cc, th * 512 : (th + 1) * 512],
                        start=(cc == 0),
                        stop=(cc == PC - 1),
                    )
                nc.vector.tensor_copy(qT_sb[:, fc, th * 512 : (th + 1) * 512], ps[:])

        # ================= Phase C: pipelined attention ================
        scale = float(HD) ** -0.5
        units = [(ih, hp) for ih in range(2) for hp in range(H // 2)]
        staged = {}
        ps_big = psS.tile([128, 4, 512], F32, tag="sbig", name="ps_big")
        slot_ctr = [0]

        def ktile(fc):
            for th in range(2):
                ps = psB.tile([128, 512], F32, tag="psB", name="psB_t")
                for cc in range(PC):
                    nc.tensor.matmul(
                        ps[:],
                        wk_sb[:, cc, fc * 128 : (fc + 1) * 128],
                        x_sb[:, cc, th * 512 : (th + 1) * 512],
                        start=(cc == 0),
                        stop=(cc == PC - 1),
                    )
                nc.vector.tensor_copy(kT_sb[:, fc, th * 512 : (th + 1) * 512], ps[:])

        def vgrp(fh, par):
            # one column-parity of the fh feature half: blocks
            # {4*fh+par, 4*fh+par+2} -> heads {8fh+2par, +1, +4, +5};
            # even blocks need only gather half 0.
            mvbase = wv_sb[:, :, fh * 512 : (fh + 1) * 512].rearrange(
                "p cc (g pr f) -> p cc g pr f", g=2, pr=2
            )
            for tc_i in range(PC):
                ps = psB.tile([128, 512], F32, tag="psB", name="psB_t")
                out_ap = ps[:, 0:256].rearrange("p (g f) -> p g f", g=2)
                for cc in range(PC):
                    nc.tensor.matmul(
                        out_ap,
                        x_sb[:, cc, tc_i * 128 : (tc_i + 1) * 128],
                        mvbase[:, cc, :, par, :],
                        start=(cc == 0),
                        stop=(cc == PC - 1),
                    )
                nc.vector.tensor_copy(
                    vp_sb[:, tc_i, :, 0:HD].rearrange(
                        "p (f g pr hh) d -> p f g pr hh d", f=2, g=2, pr=2
                    )[:, fh, :, par],
                    ps[:, 0:256].rearrange("p (g hh d) -> p g hh d", g=2, hh=2),
                )

        def stage1(u):
            ih, hp = units[u]
            i0 = ih * 512
            pts = [
                pt_p.tile([128, PC, 512], BF16, tag=f"pt{sub}", name="pt_t")
                for sub in range(2)
            ]
            for j0 in range(0, PC, 2):
                slots = []
                for sub in range(2):
                    s = slot_ctr[0] % 2
                    slot_ctr[0] += 1
                    slots.append(ps_big[:, 2 * s : 2 * s + 2, :])
                for dj in range(2):
                    for sub in range(2):  # adjacent row-group pair: concurrent
                        p0 = sub * 64
                        nc.tensor.matmul(
                            slots[sub][:, dj, :],
                            kT_sb[
                                p0 : p0 + 64,
                                hp,
                                (j0 + dj) * 128 : (j0 + dj + 1) * 128,
                            ],
                            qT_sb[p0 : p0 + 64, hp, i0 : i0 + 512],
                        )
                for sub in range(2):
                    nc.scalar.activation(
                        pts[sub][:, j0 : j0 + 2, :].rearrange("p j i -> p (j i)"),
                        slots[sub].rearrange("p j i -> p (j i)"),
                        EXP,
                        scale=scale,
                    )
            staged[u] = pts

        def stage2(u):
            ih, hp = units[u]
            i0 = ih * 512
            pts = staged.pop(u)
            for sub in range(2):
                h = 2 * hp + sub
                p0 = sub * 64
                pt = pts[sub]
                ps_o = psO.tile([HD + 1, 512], F32, tag=f"o{sub}", name="psO_t")
                for j in range(PC):
                    nc.tensor.matmul(
                        ps_o[:],
                        vp_sb[:, j, h, :],
                        pt[:, j, :],
                        start=(j == 0),
                        stop=(j == PC - 1),
                    )
                # copy psum out fast so the bank frees before the (slow)
                # broadcast/normalize chain; chain runs from SBUF
                zraw = rz_p.tile([1, 512], F32, tag="rz", name="rz_t")
                nc.vector.tensor_copy(zraw[:], ps_o[HD : HD + 1, :])
                osb = os_p.tile([HD, 512], F32, tag=f"os{sub}", name="os_t")
                nc.vector.tensor_copy(osb[:], ps_o[0:HD, :])
                zbc = zb_p.tile([HD, 512], F32, tag="zbc", name="zbc_t")
                nc.gpsimd.partition_broadcast(zbc[:], zraw[:], channels=HD)
                zb = zb_p.tile([HD, 512], F32, tag="zb", name="zb_t")
                nc.vector.reciprocal_approx_fast(zb[:], zbc[:])
                nc.vector.tensor_mul(
                    oT_sb[p0 : p0 + 64, hp, i0 : i0 + 512], osb[:], zb[:]
                )

        def proj_group(fo, th):
            ps = psB.tile([128, 512], F32, tag="psB", name="psB_t")
            for cc in range(PC):
                nc.tensor.matmul(
                    ps[:],
                    wpj_sb[:, cc, fo * 128 : (fo + 1) * 128],
                    oT_sb[:, cc, th * 512 : (th + 1) * 512],
                    start=(cc == 0),
                    stop=(cc == PC - 1),
                )
            y_sb = y_p.tile([128, 512], BF16, tag="y", name="y_t")
            nc.vector.tensor_scalar_add(y_sb[:], ps[:], bias_sb[:, fo, :])
            nc.sync.dma_start(
                out=yT_d[fo * 128 : (fo + 1) * 128, th * 512 : (th + 1) * 512],
                in_=y_sb[:],
            )

        # ---- pipelined issue order: the whole even-block universe first
        # (kT chunks 0/2/4/6, V' even column-parity, attention units with
        # even hp for both i-halves) — it depends only on gather half 0.
        # s2 trails s1 by two issue slots. ----
        useq = [0, 2, 4, 6, 8, 10, 12, 14, 1, 3, 5, 7, 9, 11, 13, 15]
        kseq = {0: 0, 1: 2, 2: 4, 3: 6, 8: 1, 9: 3, 10: 5, 11: 7}
        for i in range(16):
            if i in kseq:
                ktile(kseq[i])
            stage1(useq[i])
            if i >= 2:
                stage2(useq[i - 2])
            if i >= 13:
                proj_group(i - 13, 0)
            if i == 1:
                vgrp(0, 0)
            elif i == 2:
                vgrp(1, 0)
            elif i == 8:
                vgrp(0, 1)
            elif i == 9:
                vgrp(1, 1)
        # fill the end-of-exp-stream PE idle with the remaining th=0
        # projections (they need only ih=0 units, long since done)
        for fo in range(3, PC):
            proj_group(fo, 0)
        stage2(useq[14])
        stage2(useq[15])

        # ---- remaining projection half ----
        for fo in range(PC):
            proj_group(fo, 1)

        # LIFO release
        y_p.release()
        slabB_p.release()
        zb_p.release()
        rz_p.release()
        os_p.release()
        pt_p.release()
        ot_p.release()
        vp_p.release()
        kt_p.release()
        qt_p.release()
        psO.release()
        psS.release()
        psB.release()
        wpj_p.release()
        wv_p.release()
        wk_p.release()
        x_p.release()
        small_p.release()

    nc.compile()
    return nc


_CACHE = {}


def _get_nc():
    if "nc" not in _CACHE:
        _CACHE["nc"] = _build()
    return _CACHE["nc"]


def _host_prep(x, W_qkv, W_proj, b_proj, coef_k, coef_v, indices, task):
    import ml_dtypes

    x = np.asarray(x, dtype=np.float32)
    W_qkv = np.asarray(W_qkv, dtype=np.float32)
    W_proj = np.asarray(W_proj, dtype=np.float32)
    b_proj = np.asarray(b_proj, dtype=np.float32)
    coef_k = np.asarray(coef_k, dtype=np.float32)
    coef_v = np.asarray(coef_v, dtype=np.float32)
    indices = np.asarray(indices)
    t = int(np.asarray(task).reshape(())) + 1

    assert x.shape == (B, N, C), x.shape

    def bf(a):
        return np.ascontiguousarray(a).astype(ml_dtypes.bfloat16)

    # Host-side input marshaling: scatter the per-task frequency coefficients
    # into dense C x C planes (the sum across tasks commutes with the linear
    # inverse DCT), exactly as the reference does before its matmuls.
    def scatter(coef, idx):
        s = np.zeros(C * C, dtype=np.float32)
        np.add.at(s, idx.reshape(-1).astype(np.int64), coef.reshape(-1))
        return s.reshape(C, C)

    bm = _dct_matrix(C)
    sk = scatter(coef_k[:t], indices[:t])
    sv = scatter(coef_v[:t], indices[:t])
    wqkvT = np.ascontiguousarray(W_qkv.T)

    def sparse_buckets(s):
        # per-(partition, column-chunk) padded index/value buckets for the
        # on-device local_scatter densify; index = column within chunk
        ci, fi = np.nonzero(s)
        vals = s[ci, fi]
        p = ci % 128
        cc = ci // 128
        chunk = fi // C  # always 0; fi < C
        idx = np.full((PC, 128, SWM), -1, np.int16)
        val = np.zeros((PC, 128, SWM), np.float32)
        counts = np.zeros((PC, 128), np.int32)
        for a, b, f, v in zip(cc, p, fi, vals):
            n = counts[a, b]
            assert n < SWM, "scatter bucket overflow; raise SWM"
            idx[a, b, n] = f
            val[a, b, n] = v
            counts[a, b] = n + 1
        # layout [(cc p), m]
        return (
            np.ascontiguousarray(idx.reshape(C, SWM)),
            np.ascontiguousarray(val.reshape(C, SWM)).astype(ml_dtypes.bfloat16),
        )

    ski, skv = sparse_buckets(sk)
    svi, svv = sparse_buckets(sv)

    shared = {
        "wqkvT": bf(wqkvT),
        "wprojT": bf(W_proj.T),
        "bias": np.ascontiguousarray(b_proj.reshape(C, 1)),
        "bm": bf(bm),
    }
    maps = []
    for b in range(NCORES):
        fq = b % 4
        base = C if b < 4 else 2 * C
        maps.append(
            {
                "xT": bf(x[b].T),
                "swidx": ski if b < 4 else svi,
                "swval": skv if b < 4 else svv,
                "bmq": bf(bm[:, fq * 256 : (fq + 1) * 256]),
                "wqsl": bf(wqkvT[:, base + fq * 256 : base + (fq + 1) * 256]),
                **shared,
            }
        )
    return maps


def kernel(x, W_qkv, W_proj, b_proj, coef_k, coef_v, indices, task):
    in_maps = _host_prep(x, W_qkv, W_proj, b_proj, coef_k, coef_v, indices, task)
    nc = _get_nc()
    res = run_bass_kernel_spmd(nc, in_maps, list(range(NCORES)))

    out = np.empty((B, N, C), dtype=np.float32)
    for b in range(NCORES):
        out[b] = np.asarray(res.results[b]["yT"], dtype=np.float32).T
    return out
